# revision 13
# baseline (speedup 1.0000x reference)
"""Trainium2 Bass kernel for the BasicCae loss (encoder + contractive jac
term + decoder), SPMD across 8 NeuronCores.

Reference computation (fp32):
    y_enc = sigmoid(x @ W_enc.T + b_enc)            # [B=256, F=1500]
    s2 = (y_enc*(1-y_enc))**2
    row_norm2 = sum(W_enc**2, axis=1)               # [F]
    jac_reg = sum(s2 * row_norm2)                   # scalar
    y_out = sigmoid(y_enc @ W_dec.T + b_dec)        # [B, I=28224]
    returns (y_out, jac_reg)

Sharding (model-parallel — weights dominate traffic, so they are sharded
and the small activations replicated/gathered):
  - encoder: F padded 1500->1536, 192 features per core. Each core computes
    y_encT[f_shard, B] plus its slice of row_norm2 and of colsum(s2).
  - AllGather of y_encT (bf16) across the 8 cores.
  - decoder: I sharded 3528 per core -> y_outT[i_shard, B].
Host side only does layout prep (transpose/slice/cast) and the final
concat + 1536-element dot for jac_reg.

All matmuls run in bf16 with fp32 PSUM accumulation.
"""
import os
import numpy as np
import ml_dtypes

import concourse.bass as bass
import concourse.bacc as bacc
import concourse.tile as tile
import concourse.mybir as mybir
from concourse.bass_utils import run_bass_kernel_spmd

N_CORES = 8
I = 28224          # input size
F = 1500           # feature size
F_PAD = 1536
B = 256            # batch
F_SHARD = F_PAD // N_CORES          # 192
I_SHARD = I // N_CORES              # 3528
KI_FULL = I // 128                  # 220 full k-slices
KI_REM = I - KI_FULL * 128          # 64
KI_SLICES = KI_FULL + 1             # 221
KF_SLICES = F_PAD // 128            # 12
MD_FULL = I_SHARD // 128            # 27 full decoder M-tiles
MD_REM = I_SHARD - MD_FULL * 128    # 72
MD_TILES = MD_FULL + 1              # 28

BF16 = mybir.dt.bfloat16
F32 = mybir.dt.float32
AF = mybir.ActivationFunctionType
ALU = mybir.AluOpType

_CACHE = {}


def _build():
    # debug probes: limit encoder k-slices / skip phases (wrong numerics,
    # used only to bisect hardware-load failures by scale)
    ki_slices = int(os.environ.get("CAE_KI_SLICES", KI_SLICES))
    skip_dec = os.environ.get("CAE_SKIP_DEC", "") == "1"
    skip_rn = os.environ.get("CAE_SKIP_RN", "") == "1"
    nc = bacc.Bacc("TRN2", target_bir_lowering=False, debug=False,
                   num_devices=N_CORES)
    xT = nc.dram_tensor("xT", [I, B], BF16, kind="ExternalInput")
    wencT = nc.dram_tensor("wencT", [I, F_SHARD], BF16, kind="ExternalInput")
    benc = nc.dram_tensor("benc", [128, 2], F32, kind="ExternalInput")
    mask = nc.dram_tensor("mask", [128, 2], F32, kind="ExternalInput")
    wdecT = nc.dram_tensor("wdecT", [F_PAD, I_SHARD], BF16,
                           kind="ExternalInput")
    bdec = nc.dram_tensor("bdec", [128, MD_TILES], F32, kind="ExternalInput")
    youtT = nc.dram_tensor("youtT", [I_SHARD, B], F32, kind="ExternalOutput")
    jvec = nc.dram_tensor("jvec", [F_SHARD, 1], F32, kind="ExternalOutput")

    with tile.TileContext(nc) as tc:
        with tc.tile_pool(name="consts", bufs=1) as cpool, \
             tc.tile_pool(name="enc_stream", bufs=4) as epool, \
             tc.tile_pool(name="enc_psum", bufs=1, space="PSUM") as eps, \
             tc.tile_pool(name="dec_psum", bufs=4, space="PSUM") as dps, \
             tc.tile_pool(name="work", bufs=2) as wpool, \
             tc.tile_pool(name="dram", bufs=1, space="DRAM") as dram:

            benc_sb = cpool.tile([128, 2], F32)
            mask_sb = cpool.tile([128, 2], F32)
            bdec_sb = cpool.tile([128, MD_TILES], F32)
            nc.sync.dma_start(benc_sb[:], benc.ap())
            nc.sync.dma_start(mask_sb[:], mask.ap())
            nc.sync.dma_start(bdec_sb[:], bdec.ap())

            ones_bf = cpool.tile([128, 1], BF16)
            nc.vector.memset(ones_bf[:], 1.0)
            # rn accumulators: bf16 group acc, folded into f32 every FOLD
            # super-tiles (see fold loop below)
            J = 4
            W_E = J * F_SHARD                     # 768
            sqacc = cpool.tile([128, W_E], BF16)
            nc.vector.memset(sqacc[:], 0.0)
            acc4 = cpool.tile([128, W_E], F32)
            nc.vector.memset(acc4[:], 0.0)
            FOLD = 8

            # ---------------- encoder: y_encT[f_shard, B] ----------------
            # super-tiles of J=4 k-slices per DMA; 55 supers + 64-row tail
            n_super = min(KI_FULL, ki_slices) // J
            ye_ps0 = eps.tile([128, B], F32)
            ye_ps1 = eps.tile([128, B], F32)
            for s in range(n_super):
                r0 = s * J * 128
                xt = epool.tile([128, J * B], BF16, tag="xt")
                wt = epool.tile([128, W_E], BF16, tag="wt")
                nc.sync.dma_start(
                    xt.rearrange("p (j b) -> p j b", j=J),
                    xT.ap()[r0:r0 + J * 128, :]
                    .rearrange("(j p) b -> p j b", p=128))
                nc.sync.dma_start(
                    wt.rearrange("p (j f) -> p j f", j=J),
                    wencT.ap()[r0:r0 + J * 128, :]
                    .rearrange("(j p) f -> p j f", p=128))
                for j in range(J):
                    k = s * J + j
                    nc.tensor.matmul(ye_ps0[:, :],
                                     wt[:, j * F_SHARD:j * F_SHARD + 128],
                                     xt[:, j * B:(j + 1) * B],
                                     start=(k == 0), stop=False)
                    nc.tensor.matmul(ye_ps1[0:64, :],
                                     wt[:, j * F_SHARD + 128:
                                        (j + 1) * F_SHARD],
                                     xt[:, j * B:(j + 1) * B],
                                     start=(k == 0), stop=False)
                if not skip_rn:
                    sq = epool.tile([128, W_E], BF16, tag="sq")
                    nc.vector.tensor_mul(sq[:], wt[:], wt[:])
                    nc.vector.tensor_add(sqacc[:], sqacc[:], sq[:])
                    if (s + 1) % FOLD == 0 or s == n_super - 1:
                        nc.vector.tensor_add(acc4[:], acc4[:], sqacc[:])
                        nc.vector.memset(sqacc[:], 0.0)
            # tail k-slice (64 rows)
            r0 = n_super * J * 128
            ksz = KI_REM
            xt_t = epool.tile([128, B], BF16, tag="xt_t")
            wt_t = epool.tile([128, F_SHARD], BF16, tag="wt_t")
            nc.sync.dma_start(xt_t[:ksz, :], xT.ap()[r0:r0 + ksz, :])
            nc.sync.dma_start(wt_t[:ksz, :], wencT.ap()[r0:r0 + ksz, :])
            nc.tensor.matmul(ye_ps0[:, :], wt_t[:ksz, 0:128], xt_t[:ksz, :],
                             start=False, stop=True)
            nc.tensor.matmul(ye_ps1[0:64, :], wt_t[:ksz, 128:192],
                             xt_t[:ksz, :], start=False, stop=True)
            if not skip_rn:
                sq_t = epool.tile([128, F_SHARD], BF16, tag="sq_t")
                nc.vector.tensor_mul(sq_t[:ksz, :], wt_t[:ksz, :],
                                     wt_t[:ksz, :])
                nc.vector.tensor_add(acc4[:ksz, 0:F_SHARD],
                                     acc4[:ksz, 0:F_SHARD], sq_t[:ksz, :])
                # collapse the 4 column groups into group 0
                for g in range(1, J):
                    nc.vector.tensor_add(
                        acc4[:, 0:F_SHARD], acc4[:, 0:F_SHARD],
                        acc4[:, g * F_SHARD:(g + 1) * F_SHARD])
                rn_bf = cpool.tile([128, F_SHARD], BF16)
                nc.vector.tensor_copy(rn_bf[:], acc4[:, 0:F_SHARD])

            # sigmoid(+bias), jac pieces, masked bf16 cast, AG bounce
            ag_in = dram.tile([F_SHARD, B], BF16)
            ag_out = dram.tile([F_PAD, B], BF16)
            cs_list = []
            for m, (ps, msz) in enumerate(((ye_ps0, 128), (ye_ps1, 64))):
                y = wpool.tile([128, B], F32, tag="y")
                nc.scalar.activation(y[:msz, :], ps[:msz, :], AF.Sigmoid,
                                     bias=benc_sb[:msz, m:m + 1])
                ybf = wpool.tile([128, B], BF16, tag="ybf")
                nc.vector.tensor_scalar_mul(ybf[:msz, :], y[:msz, :],
                                            mask_sb[:msz, m:m + 1])
                nc.sync.dma_start(ag_in[m * 128:m * 128 + msz, :],
                                  ybf[:msz, :])
                y2 = wpool.tile([128, B], F32, tag="y2")
                nc.vector.tensor_mul(y2[:msz, :], y[:msz, :], y[:msz, :])
                u = wpool.tile([128, B], F32, tag="u")
                nc.vector.tensor_sub(u[:msz, :], y[:msz, :], y2[:msz, :])
                s2 = wpool.tile([128, B], F32, tag="s2")
                nc.vector.tensor_mul(s2[:msz, :], u[:msz, :], u[:msz, :])
                cs = cpool.tile([128, 1], F32, tag=f"cs{m}")
                nc.vector.tensor_reduce(cs[:msz, :], s2[:msz, :],
                                        mybir.AxisListType.X, ALU.add)
                cs_list.append(cs)

            nc.gpsimd.collective_compute(
                "AllGather", ALU.bypass,
                replica_groups=[list(range(N_CORES))],
                ins=[ag_in.opt()], outs=[ag_out.opt()],
            )

            # jvec = cs * row_norm2 (column reduce of rn_acc via PE)
            for m, msz in ((0, 128), (1, 64)):
                jv = wpool.tile([128, 1], F32, tag="jv")
                if skip_rn:
                    nc.vector.memset(jv[:msz, :], 0.0)
                else:
                    rn_ps = dps.tile([128, 1], F32, tag="rn_ps", bufs=2)
                    nc.tensor.matmul(rn_ps[:msz, :],
                                     rn_bf[:, m * 128:m * 128 + msz],
                                     ones_bf[:, :], start=True, stop=True)
                    nc.vector.tensor_mul(jv[:msz, :], cs_list[m][:msz, :],
                                         rn_ps[:msz, :])
                nc.sync.dma_start(jvec.ap()[m * 128:m * 128 + msz, :],
                                  jv[:msz, :])

            # ---------------- decoder: y_outT[i_shard, B] ----------------
            ye_sb = []
            wd_sb = []
            if not skip_dec:
                for kf in range(KF_SLICES):
                    yt = cpool.tile([128, B], BF16, tag=f"ye{kf}",
                                    name=f"ye{kf}")
                    nc.sync.dma_start(yt[:],
                                      ag_out[kf * 128:(kf + 1) * 128, :])
                    ye_sb.append(yt)
                for kf in range(KF_SLICES):
                    wd = cpool.tile([128, I_SHARD], BF16, tag=f"wd{kf}",
                                    name=f"wd{kf}")
                    nc.sync.dma_start(wd[:],
                                      wdecT.ap()[kf * 128:(kf + 1) * 128, :])
                    wd_sb.append(wd)

            for m in range(MD_TILES):
                msz = 128 if m < MD_FULL else MD_REM
                c0 = m * 128
                if skip_dec:
                    yo = wpool.tile([128, B], F32, tag="yo")
                    nc.vector.memset(yo[:msz, :], 0.0)
                    nc.sync.dma_start(youtT.ap()[c0:c0 + msz, :], yo[:msz, :])
                    continue
                ps = dps.tile([128, B], F32, tag="dps")
                for kf in range(KF_SLICES):
                    nc.tensor.matmul(ps[:msz, :],
                                     wd_sb[kf][:, c0:c0 + msz],
                                     ye_sb[kf][:, :],
                                     start=(kf == 0),
                                     stop=(kf == KF_SLICES - 1))
                yo = wpool.tile([128, B], F32, tag="yo")
                nc.scalar.activation(yo[:msz, :], ps[:msz, :], AF.Sigmoid,
                                     bias=bdec_sb[:msz, m:m + 1])
                nc.sync.dma_start(youtT.ap()[c0:c0 + msz, :], yo[:msz, :])
    nc.compile()
    return nc


def _prep_inputs(x, W_enc, b_enc, W_dec, b_dec):
    bf16 = ml_dtypes.bfloat16
    xT = np.ascontiguousarray(x.astype(bf16).T)               # [I, B]
    wenc_bf = W_enc.astype(bf16)                              # [F, I]
    wdec_bf = W_dec.astype(bf16)                              # [I, F]

    benc_pad = np.zeros(F_PAD, np.float32)
    benc_pad[:F] = b_enc

    in_maps = []
    for c in range(N_CORES):
        f0 = c * F_SHARD
        f1 = min(F, f0 + F_SHARD)
        nf = max(0, f1 - f0)
        wencT = np.zeros((I, F_SHARD), bf16)
        if nf > 0:
            wencT[:, :nf] = wenc_bf[f0:f1, :].T
        benc_loc = np.zeros(256, np.float32)
        benc_loc[:F_SHARD] = benc_pad[f0:f0 + F_SHARD]
        benc_c = np.ascontiguousarray(benc_loc.reshape(2, 128).T)
        maskv = np.zeros(256, np.float32)
        maskv[:nf] = 1.0
        mask_c = np.ascontiguousarray(maskv.reshape(2, 128).T)

        i0 = c * I_SHARD
        wdecT = np.zeros((F_PAD, I_SHARD), bf16)
        wdecT[:F, :] = wdec_bf[i0:i0 + I_SHARD, :].T
        bdec_loc = np.zeros(MD_TILES * 128, np.float32)
        bdec_loc[:I_SHARD] = b_dec[i0:i0 + I_SHARD]
        bdec_c = np.ascontiguousarray(bdec_loc.reshape(MD_TILES, 128).T)
        in_maps.append({
            "xT": xT,
            "wencT": wencT,
            "benc": benc_c,
            "mask": mask_c,
            "wdecT": wdecT,
            "bdec": bdec_c,
        })
    return in_maps


def run(x, W_enc, b_enc, W_dec, b_dec, trace=False, **trace_kwargs):
    if "nc" not in _CACHE:
        _CACHE["nc"] = _build()
    nc = _CACHE["nc"]
    in_maps = _prep_inputs(np.asarray(x, np.float32),
                           np.asarray(W_enc, np.float32),
                           np.asarray(b_enc, np.float32),
                           np.asarray(W_dec, np.float32),
                           np.asarray(b_dec, np.float32))
    res = run_bass_kernel_spmd(nc, in_maps, core_ids=list(range(N_CORES)),
                               trace=trace, **trace_kwargs)
    youtT = np.concatenate([res.results[c]["youtT"] for c in range(N_CORES)],
                           axis=0)                            # [I, B]
    y_out = np.ascontiguousarray(youtT.T)                     # [B, I]
    jac = np.float32(sum(res.results[c]["jvec"].sum(dtype=np.float64)
                         for c in range(N_CORES)))
    return (y_out, np.asarray(jac, np.float32)), res


def kernel(x, W_enc, b_enc, W_dec, b_dec):
    (y_out, jac), _ = run(x, W_enc, b_enc, W_dec, b_dec)
    return y_out, jac


# revision 15
# speedup vs baseline: 3.5684x; 3.5684x over previous
"""Trainium2 Bass kernel for the BasicCae loss (encoder + contractive jac
term + decoder), SPMD across 8 NeuronCores.

Reference computation (fp32):
    y_enc = sigmoid(x @ W_enc.T + b_enc)            # [B=256, F=1500]
    s2 = (y_enc*(1-y_enc))**2
    row_norm2 = sum(W_enc**2, axis=1)               # [F]
    jac_reg = sum(s2 * row_norm2)                   # scalar
    y_out = sigmoid(y_enc @ W_dec.T + b_dec)        # [B, I=28224]
    returns (y_out, jac_reg)

Sharding (model-parallel — weights dominate traffic, so they are sharded
and the small activations replicated/gathered):
  - encoder: F padded 1500->1536, 192 features per core. Each core computes
    y_encT[f_shard, B] plus its slice of row_norm2 and of colsum(s2).
  - AllGather of y_encT (bf16) across the 8 cores.
  - decoder: I sharded 3528 per core -> y_outT[i_shard, B].
Host side only does layout prep (transpose/slice/cast) and the final
concat + 1536-element dot for jac_reg.

All matmuls run in bf16 with fp32 PSUM accumulation.
"""
import os
import numpy as np
import ml_dtypes

import concourse.bass as bass
import concourse.bacc as bacc
import concourse.tile as tile
import concourse.mybir as mybir
from concourse.bass_utils import run_bass_kernel_spmd

N_CORES = 8
I = 28224          # input size
F = 1500           # feature size
F_PAD = 1536
B = 256            # batch
F_SHARD = F_PAD // N_CORES          # 192
I_SHARD = I // N_CORES              # 3528
KI_FULL = I // 128                  # 220 full k-slices
KI_REM = I - KI_FULL * 128          # 64
KI_SLICES = KI_FULL + 1             # 221
KF_SLICES = F_PAD // 128            # 12
MD_FULL = I_SHARD // 128            # 27 full decoder M-tiles
MD_REM = I_SHARD - MD_FULL * 128    # 72
MD_TILES = MD_FULL + 1              # 28

BF16 = mybir.dt.bfloat16
F32 = mybir.dt.float32
AF = mybir.ActivationFunctionType
ALU = mybir.AluOpType

_CACHE = {}


def _build():
    # debug probes: limit encoder k-slices / skip phases (wrong numerics,
    # used only to bisect hardware-load failures by scale)
    ki_slices = int(os.environ.get("CAE_KI_SLICES", KI_SLICES))
    skip_dec = os.environ.get("CAE_SKIP_DEC", "") == "1"
    skip_rn = os.environ.get("CAE_SKIP_RN", "") == "1"
    nc = bacc.Bacc("TRN2", target_bir_lowering=False, debug=False,
                   num_devices=N_CORES)
    xT = nc.dram_tensor("xT", [I, B], BF16, kind="ExternalInput")
    wencT = nc.dram_tensor("wencT", [I, F_SHARD], BF16, kind="ExternalInput")
    benc = nc.dram_tensor("benc", [128, 2], F32, kind="ExternalInput")
    mask = nc.dram_tensor("mask", [128, 2], F32, kind="ExternalInput")
    wdecT = nc.dram_tensor("wdecT", [F_PAD, I_SHARD], BF16,
                           kind="ExternalInput")
    bdec = nc.dram_tensor("bdec", [128, MD_TILES], F32, kind="ExternalInput")
    youtT = nc.dram_tensor("youtT", [I_SHARD, B], F32, kind="ExternalOutput")
    jvec = nc.dram_tensor("jvec", [F_SHARD, 1], F32, kind="ExternalOutput")

    with tile.TileContext(nc) as tc:
        with tc.tile_pool(name="consts", bufs=1) as cpool, \
             tc.tile_pool(name="enc_stream", bufs=4) as epool, \
             tc.tile_pool(name="enc_psum", bufs=1, space="PSUM") as eps, \
             tc.tile_pool(name="dec_psum", bufs=4, space="PSUM") as dps, \
             tc.tile_pool(name="work", bufs=2) as wpool, \
             tc.tile_pool(name="dram", bufs=1, space="DRAM") as dram:

            benc_sb = cpool.tile([128, 2], F32)
            mask_sb = cpool.tile([128, 2], F32)
            bdec_sb = cpool.tile([128, MD_TILES], F32)
            nc.sync.dma_start(benc_sb[:], benc.ap())
            nc.sync.dma_start(mask_sb[:], mask.ap())
            nc.sync.dma_start(bdec_sb[:], bdec.ap())

            ones_bf = cpool.tile([128, 1], BF16)
            nc.vector.memset(ones_bf[:], 1.0)
            # rn accumulators: bf16 group acc, folded into f32 every FOLD
            # super-tiles (see fold loop below)
            J = 4
            W_E = J * F_SHARD                     # 768
            sqacc = cpool.tile([128, W_E], BF16)
            nc.vector.memset(sqacc[:], 0.0)
            acc4 = cpool.tile([128, W_E], F32)
            nc.vector.memset(acc4[:], 0.0)
            FOLD = 8

            # PE warm-up: ~4us of dummy matmuls so HAM releases the clock
            # gate before the real stream arrives (scratch PSUM, zero data)
            if os.environ.get("CAE_NO_WARMUP", "") != "1":
                warm_w = cpool.tile([128, 128], BF16)
                warm_x = cpool.tile([128, B], BF16)
                nc.vector.memset(warm_w[:], 0.0)
                nc.vector.memset(warm_x[:], 0.0)
                for _ in range(20):
                    wps = dps.tile([128, B], F32, tag="dps")
                    nc.tensor.matmul(wps[:, :], warm_w[:, :], warm_x[:, :],
                                     start=True, stop=True)

            # ---------------- encoder: y_encT[f_shard, B] ----------------
            # super-tiles of J=4 k-slices per DMA; 55 supers + 64-row tail
            n_super = min(KI_FULL, ki_slices) // J
            ye_ps0 = eps.tile([128, B], F32)
            ye_ps1 = eps.tile([128, B], F32)
            for s in range(n_super):
                r0 = s * J * 128
                xt = epool.tile([128, J * B], BF16, tag="xt")
                wt = epool.tile([128, W_E], BF16, tag="wt")
                # interleaved contraction map: partition p of slice j holds
                # DRAM row r0+J*p+j, so each partition's load is J
                # consecutive rows = one contiguous 2KB burst. The i->(p,j)
                # bijection is identical for both operands, so the matmul
                # accumulation is unchanged.
                nc.sync.dma_start(
                    xt[:, :],
                    xT.ap()[r0:r0 + J * 128, :]
                    .rearrange("(p j) b -> p (j b)", j=J))
                nc.sync.dma_start(
                    wt[:, :],
                    wencT.ap()[r0:r0 + J * 128, :]
                    .rearrange("(p j) f -> p (j f)", j=J))
                for j in range(J):
                    k = s * J + j
                    nc.tensor.matmul(ye_ps0[:, :],
                                     wt[:, j * F_SHARD:j * F_SHARD + 128],
                                     xt[:, j * B:(j + 1) * B],
                                     start=(k == 0), stop=False)
                    nc.tensor.matmul(ye_ps1[0:64, :],
                                     wt[:, j * F_SHARD + 128:
                                        (j + 1) * F_SHARD],
                                     xt[:, j * B:(j + 1) * B],
                                     start=(k == 0), stop=False)
                if not skip_rn:
                    sq = epool.tile([128, W_E], BF16, tag="sq")
                    nc.vector.tensor_mul(sq[:], wt[:], wt[:])
                    nc.vector.tensor_add(sqacc[:], sqacc[:], sq[:])
                    if (s + 1) % FOLD == 0 or s == n_super - 1:
                        nc.vector.tensor_add(acc4[:], acc4[:], sqacc[:])
                        nc.vector.memset(sqacc[:], 0.0)
            # tail k-slice (64 rows)
            r0 = n_super * J * 128
            ksz = KI_REM
            xt_t = epool.tile([128, B], BF16, tag="xt_t")
            wt_t = epool.tile([128, F_SHARD], BF16, tag="wt_t")
            nc.sync.dma_start(xt_t[:ksz, :], xT.ap()[r0:r0 + ksz, :])
            nc.sync.dma_start(wt_t[:ksz, :], wencT.ap()[r0:r0 + ksz, :])
            nc.tensor.matmul(ye_ps0[:, :], wt_t[:ksz, 0:128], xt_t[:ksz, :],
                             start=False, stop=True)
            nc.tensor.matmul(ye_ps1[0:64, :], wt_t[:ksz, 128:192],
                             xt_t[:ksz, :], start=False, stop=True)
            if not skip_rn:
                sq_t = epool.tile([128, F_SHARD], BF16, tag="sq_t")
                nc.vector.tensor_mul(sq_t[:ksz, :], wt_t[:ksz, :],
                                     wt_t[:ksz, :])
                nc.vector.tensor_add(acc4[:ksz, 0:F_SHARD],
                                     acc4[:ksz, 0:F_SHARD], sq_t[:ksz, :])
                # collapse the 4 column groups into group 0
                for g in range(1, J):
                    nc.vector.tensor_add(
                        acc4[:, 0:F_SHARD], acc4[:, 0:F_SHARD],
                        acc4[:, g * F_SHARD:(g + 1) * F_SHARD])
                rn_bf = cpool.tile([128, F_SHARD], BF16)
                nc.vector.tensor_copy(rn_bf[:], acc4[:, 0:F_SHARD])

            # sigmoid(+bias), jac pieces, masked bf16 cast, AG bounce
            ag_in = dram.tile([F_SHARD, B], BF16)
            ag_out = dram.tile([F_PAD, B], BF16)
            cs_list = []
            for m, (ps, msz) in enumerate(((ye_ps0, 128), (ye_ps1, 64))):
                y = wpool.tile([128, B], F32, tag="y")
                nc.scalar.activation(y[:msz, :], ps[:msz, :], AF.Sigmoid,
                                     bias=benc_sb[:msz, m:m + 1])
                ybf = wpool.tile([128, B], BF16, tag="ybf")
                nc.vector.tensor_scalar_mul(ybf[:msz, :], y[:msz, :],
                                            mask_sb[:msz, m:m + 1])
                nc.sync.dma_start(ag_in[m * 128:m * 128 + msz, :],
                                  ybf[:msz, :])
                y2 = wpool.tile([128, B], F32, tag="y2")
                nc.vector.tensor_mul(y2[:msz, :], y[:msz, :], y[:msz, :])
                u = wpool.tile([128, B], F32, tag="u")
                nc.vector.tensor_sub(u[:msz, :], y[:msz, :], y2[:msz, :])
                s2 = wpool.tile([128, B], F32, tag="s2")
                nc.vector.tensor_mul(s2[:msz, :], u[:msz, :], u[:msz, :])
                cs = cpool.tile([128, 1], F32, tag=f"cs{m}")
                nc.vector.tensor_reduce(cs[:msz, :], s2[:msz, :],
                                        mybir.AxisListType.X, ALU.add)
                cs_list.append(cs)

            nc.gpsimd.collective_compute(
                "AllGather", ALU.bypass,
                replica_groups=[list(range(N_CORES))],
                ins=[ag_in.opt()], outs=[ag_out.opt()],
            )

            # jvec = cs * row_norm2 (column reduce of rn_acc via PE)
            for m, msz in ((0, 128), (1, 64)):
                jv = wpool.tile([128, 1], F32, tag="jv")
                if skip_rn:
                    nc.vector.memset(jv[:msz, :], 0.0)
                else:
                    rn_ps = dps.tile([128, 1], F32, tag="rn_ps", bufs=2)
                    nc.tensor.matmul(rn_ps[:msz, :],
                                     rn_bf[:, m * 128:m * 128 + msz],
                                     ones_bf[:, :], start=True, stop=True)
                    nc.vector.tensor_mul(jv[:msz, :], cs_list[m][:msz, :],
                                         rn_ps[:msz, :])
                nc.sync.dma_start(jvec.ap()[m * 128:m * 128 + msz, :],
                                  jv[:msz, :])

            # ---------------- decoder: y_outT[i_shard, B] ----------------
            ye_sb = []
            wd_sb = []
            if not skip_dec:
                for kf in range(KF_SLICES):
                    yt = cpool.tile([128, B], BF16, tag=f"ye{kf}",
                                    name=f"ye{kf}")
                    nc.sync.dma_start(yt[:],
                                      ag_out[kf * 128:(kf + 1) * 128, :])
                    ye_sb.append(yt)
                for kf in range(KF_SLICES):
                    wd = cpool.tile([128, I_SHARD], BF16, tag=f"wd{kf}",
                                    name=f"wd{kf}")
                    nc.sync.dma_start(wd[:],
                                      wdecT.ap()[kf * 128:(kf + 1) * 128, :])
                    wd_sb.append(wd)

            for m in range(MD_TILES):
                msz = 128 if m < MD_FULL else MD_REM
                c0 = m * 128
                if skip_dec:
                    yo = wpool.tile([128, B], F32, tag="yo")
                    nc.vector.memset(yo[:msz, :], 0.0)
                    nc.sync.dma_start(youtT.ap()[c0:c0 + msz, :], yo[:msz, :])
                    continue
                ps = dps.tile([128, B], F32, tag="dps")
                for kf in range(KF_SLICES):
                    nc.tensor.matmul(ps[:msz, :],
                                     wd_sb[kf][:, c0:c0 + msz],
                                     ye_sb[kf][:, :],
                                     start=(kf == 0),
                                     stop=(kf == KF_SLICES - 1))
                yo = wpool.tile([128, B], F32, tag="yo")
                nc.scalar.activation(yo[:msz, :], ps[:msz, :], AF.Sigmoid,
                                     bias=bdec_sb[:msz, m:m + 1])
                nc.sync.dma_start(youtT.ap()[c0:c0 + msz, :], yo[:msz, :])
    nc.compile()
    return nc


def _prep_inputs(x, W_enc, b_enc, W_dec, b_dec):
    bf16 = ml_dtypes.bfloat16
    xT = np.ascontiguousarray(x.astype(bf16).T)               # [I, B]
    wenc_bf = W_enc.astype(bf16)                              # [F, I]
    wdec_bf = W_dec.astype(bf16)                              # [I, F]

    benc_pad = np.zeros(F_PAD, np.float32)
    benc_pad[:F] = b_enc

    in_maps = []
    for c in range(N_CORES):
        f0 = c * F_SHARD
        f1 = min(F, f0 + F_SHARD)
        nf = max(0, f1 - f0)
        wencT = np.zeros((I, F_SHARD), bf16)
        if nf > 0:
            wencT[:, :nf] = wenc_bf[f0:f1, :].T
        benc_loc = np.zeros(256, np.float32)
        benc_loc[:F_SHARD] = benc_pad[f0:f0 + F_SHARD]
        benc_c = np.ascontiguousarray(benc_loc.reshape(2, 128).T)
        maskv = np.zeros(256, np.float32)
        maskv[:nf] = 1.0
        mask_c = np.ascontiguousarray(maskv.reshape(2, 128).T)

        i0 = c * I_SHARD
        wdecT = np.zeros((F_PAD, I_SHARD), bf16)
        wdecT[:F, :] = wdec_bf[i0:i0 + I_SHARD, :].T
        bdec_loc = np.zeros(MD_TILES * 128, np.float32)
        bdec_loc[:I_SHARD] = b_dec[i0:i0 + I_SHARD]
        bdec_c = np.ascontiguousarray(bdec_loc.reshape(MD_TILES, 128).T)
        in_maps.append({
            "xT": xT,
            "wencT": wencT,
            "benc": benc_c,
            "mask": mask_c,
            "wdecT": wdecT,
            "bdec": bdec_c,
        })
    return in_maps


def run(x, W_enc, b_enc, W_dec, b_dec, trace=False, **trace_kwargs):
    if "nc" not in _CACHE:
        _CACHE["nc"] = _build()
    nc = _CACHE["nc"]
    in_maps = _prep_inputs(np.asarray(x, np.float32),
                           np.asarray(W_enc, np.float32),
                           np.asarray(b_enc, np.float32),
                           np.asarray(W_dec, np.float32),
                           np.asarray(b_dec, np.float32))
    res = run_bass_kernel_spmd(nc, in_maps, core_ids=list(range(N_CORES)),
                               trace=trace, **trace_kwargs)
    youtT = np.concatenate([res.results[c]["youtT"] for c in range(N_CORES)],
                           axis=0)                            # [I, B]
    y_out = np.ascontiguousarray(youtT.T)                     # [B, I]
    jac = np.float32(sum(res.results[c]["jvec"].sum(dtype=np.float64)
                         for c in range(N_CORES)))
    return (y_out, np.asarray(jac, np.float32)), res


def kernel(x, W_enc, b_enc, W_dec, b_dec):
    (y_out, jac), _ = run(x, W_enc, b_enc, W_dec, b_dec)
    return y_out, jac
